# revision 11
# baseline (speedup 1.0000x reference)
"""Trainium2 Bass kernel for nn_Convolution_24970939858998.

Conv2d: input [32, 8, 1024, 1024] f32, weight [8, 8, 3, 3], bias [8],
stride 1, pad 1 -> out [32, 8, 1024, 1024].

Strategy
--------
Data-parallel over batch: 4 images per core x 8 cores, no collectives.

Per core, the conv is computed as a *banded matmul*: for a block of 14
output rows, the 16 needed input rows (8 channels each -> K = 128 SBUF
partitions, p = r*8+ci) are multiplied by a host-prebuilt band weight
matrix lhsT[kw] of shape [128, 112] (m = dh*8+co, entry W[co,ci,r-dh,kw])
so a single PE pass produces all 8 output channels x 14 rows at once.
The kw=0..2 taps are 3 PSUM-accumulated matmuls whose rhs is the same
SBUF tile shifted by one column. Matmuls run in float32r (full-rate fp32
on the PE). PSUM -> SBUF copy fuses the per-channel bias add (alternating
ScalarE activation / VectorE tensor_scalar).

Host-side the input is zero-padded and laid out [hp, c, b, w] (and the
output [hop, co, b, w]) so each 16-row block is a single fully
contiguous [128 partitions x 4104] DMA, and each 14-row output strip a
contiguous [112 x 4096] DMA.
"""

import os
import sys

import numpy as np

for _p in ("/opt/trn_rl_repo", "/root/.axon_site/_ro/trn_rl_repo"):
    if os.path.isdir(_p) and _p not in sys.path:
        sys.path.insert(0, _p)
        break

import concourse.mybir as mybir
from concourse import bacc, bass_utils
from concourse.tile import TileContext

B, C, CO, H, W = 32, 8, 8, 1024, 1024
KH = KW = 3
NCORES = 8
BPC = B // NCORES  # 4 images per core

RB = 14  # output rows per block
KR = 16  # input rows per block (RB + 2 halo)
M = CO * RB  # 112 output partitions (dh*8+co)
NW = 512  # w chunk (one PSUM bank of f32)

_PROG = None  # cached traced+compiled program
LAST_RESULTS = None  # bass_utils.BassKernelResults of the last run


def geometry(h, w):
    nblk = -(-h // RB)
    hop = nblk * RB  # padded output rows
    hp = 1 + hop + (KR - RB)  # padded input rows
    wp = w + 2  # padded input cols
    return nblk, hop, hp, wp


def build_program(bpc=BPC, h=H, w=W):
    f32 = mybir.dt.float32
    f32r = mybir.dt.float32r
    nblk, hop, hp, wp = geometry(h, w)
    nch = w // NW

    nc = bacc.Bacc("TRN2", debug=False)
    # input padded + transposed on host: x[hp, c, b, wp]. float32r end-to-end
    # (same bits as f32) so the BIR verifier sees rounded producers for the
    # full-rate FP32R matmuls.
    x = nc.dram_tensor("x", [hp, C, bpc, wp], f32r, kind="ExternalInput").ap()
    wband = nc.dram_tensor("wband", [8 * KR, KW, M], f32r, kind="ExternalInput").ap()
    bias = nc.dram_tensor("bias", [M, 1], f32, kind="ExternalInput").ap()
    # output layout out[hop, co, b, w]
    out = nc.dram_tensor("out", [hop, CO, bpc, w], f32, kind="ExternalOutput").ap()

    with TileContext(nc) as tc:
        with (
            tc.tile_pool(name="const", bufs=1) as cpool,
            tc.tile_pool(name="xin", bufs=3) as xpool,
            tc.tile_pool(name="yout", bufs=3) as ypool,
            tc.tile_pool(name="acc", bufs=6, space="PSUM") as ppool,
        ):
            wt = cpool.tile([8 * KR, KW, M], f32r)
            nc.sync.dma_start(out=wt, in_=wband)
            bt = cpool.tile([M, 1], f32)
            nc.sync.dma_start(out=bt, in_=bias)

            for j in range(nblk):
                h0 = j * RB
                xt = xpool.tile([8 * KR, bpc, wp], f32r, tag="xt")
                nc.sync.dma_start(
                    out=xt, in_=x[h0 : h0 + KR].rearrange("r c b w -> (r c) b w")
                )
                yt = ypool.tile([M, bpc, w], f32, tag="yt")
                for b in range(bpc):
                    for wc in range(nch):
                        w0 = wc * NW
                        ps = ppool.tile([M, NW], f32, tag="ps")
                        for i, kw in enumerate((1, 0, 2)):
                            nc.tensor.matmul(
                                ps,
                                wt[:, kw, :],
                                xt[:, b, w0 + kw : w0 + kw + NW],
                                start=(i == 0),
                                stop=(i == 2),
                            )
                        ysec = yt[:, b, w0 : w0 + NW]
                        if (b + wc) % 2 == 0:
                            nc.scalar.add(ysec, ps, bt)
                        else:
                            nc.vector.tensor_scalar_add(ysec, ps, bt)
                nc.sync.dma_start(
                    out=out[h0 : h0 + RB].rearrange("r c b w -> (r c) b w"), in_=yt
                )
    nc.compile()
    return nc


def pack_weights(weight: np.ndarray) -> np.ndarray:
    # lhsT[r*8+ci, kw, dh*8+co] = weight[co, ci, r-dh, kw] for 0 <= r-dh < 3
    wb = np.zeros((8 * KR, KW, M), np.float32)
    for dh in range(RB):
        for kh in range(KH):
            r = dh + kh
            wb[r * 8 : r * 8 + 8, :, dh * 8 : dh * 8 + 8] = weight[
                :, :, kh, :
            ].transpose(1, 2, 0)
    return wb


def pad_input(input, h, w):
    """input [n, C, h, w] -> padded [hp, C, n, wp]."""
    nblk, hop, hp, wp = geometry(h, w)
    n = input.shape[0]
    xpad = np.zeros((hp, C, n, wp), np.float32)
    xpad[1 : 1 + h, :, :, 1 : 1 + w] = input.transpose(2, 1, 0, 3)
    return xpad


def kernel(input, weight, bias):
    global _PROG, LAST_RESULTS
    input = np.asarray(input, dtype=np.float32)
    weight = np.asarray(weight, dtype=np.float32)
    bias = np.asarray(bias, dtype=np.float32)

    if _PROG is None:
        _PROG = build_program()
    nc = _PROG

    wb = pack_weights(weight)
    bias_m = np.tile(bias.astype(np.float32), RB).reshape(M, 1)

    in_maps = [
        {
            "x": pad_input(input[c * BPC : (c + 1) * BPC], H, W),
            "wband": wb,
            "bias": bias_m,
        }
        for c in range(NCORES)
    ]
    LAST_RESULTS = bass_utils.run_bass_kernel_spmd(
        nc, in_maps, core_ids=list(range(NCORES))
    )
    # out[hop, co, b, w] -> [b, co, h, w]
    outs = [
        r["out"][:H].transpose(2, 1, 0, 3) for r in LAST_RESULTS.results
    ]
    return np.concatenate(outs, axis=0)
